# revision 28
# baseline (speedup 1.0000x reference)
"""Multi-head causal self-attention (B=2, S=2048, D=1024, H=16) on 8 TRN2 NeuronCores.

Sharding: data-parallel over batch (2) x tensor-parallel over heads (4 groups of
4 heads). Each core computes Q/K/V projections for its 4 heads, causal
flash-style attention (scores kept transposed [k, q] so no on-chip transposes
are needed), and a partial output projection against its row-slice of W_O.
Host sums the 4 partials per batch and adds the output bias.

v3: full-bf16 compute (fp32 PSUM accumulation), end-to-end rel err ~5e-3.
Causal masking happens AFTER exp as a bf16 multiply on DVE. Softmax
denominators come from an extra all-ones column appended to V; 1/den via
reciprocal_approx_fast and broadcast across partitions with a tiny PE matmul.
To keep the PE dense (HAM stays warm), the m=1 Q/K projection chunks are
interleaved group-by-group into the first attention block, and each block's
output projection is emitted one attention block late. Output partials are
written bf16 and summed on host in fp32.
"""

import contextlib
import sys

import numpy as np

sys.path.insert(0, "/opt/trn_rl_repo")

import concourse.bass as bass  # noqa: E402
import concourse.tile as tile  # noqa: E402
from concourse import bacc, mybir  # noqa: E402
from concourse.bass_utils import run_bass_kernel_spmd  # noqa: E402

F32 = mybir.dt.float32
F32R = mybir.dt.float32r
BF16 = mybir.dt.bfloat16
AF = mybir.ActivationFunctionType
ALU = mybir.AluOpType

B, S, D, H = 2, 2048, 1024, 16
DH = D // H          # 64
TPG = 4              # tensor-parallel groups
HPC = H // TPG       # 4 heads per core
CH = HPC * DH        # 256 channels per core
CHA = CH + HPC       # 260: V channels augmented with a ones column per head
N_CORES = 8

NQ = S // 512    # 4 q-blocks of 512
NT = S // 128    # 16 s-tiles / k-blocks

_PROG = None  # cached compiled Bass program


def _build_program():
    nc = bacc.Bacc("TRN2", target_bir_lowering=False, debug=False,
                   num_devices=N_CORES)

    xT = nc.dram_tensor("xT", [D, S], BF16, kind="ExternalInput").ap()
    wq = nc.dram_tensor("wq", [D, CH], BF16, kind="ExternalInput").ap()
    wk = nc.dram_tensor("wk", [D, CH], BF16, kind="ExternalInput").ap()
    wv = nc.dram_tensor("wv", [D, CHA], BF16, kind="ExternalInput").ap()
    wo = nc.dram_tensor("wo", [CH, D], BF16, kind="ExternalInput").ap()
    bq = nc.dram_tensor("bq", [128, 2], F32, kind="ExternalInput").ap()
    bk = nc.dram_tensor("bk", [128, 2], F32, kind="ExternalInput").ap()
    bv = nc.dram_tensor("bv", [1, CHA], BF16, kind="ExternalInput").ap()
    ones1 = nc.dram_tensor("ones1", [1, 128], BF16, kind="ExternalInput").ap()
    onesf = nc.dram_tensor("onesf", [1, 64], F32, kind="ExternalInput").ap()
    maskb = nc.dram_tensor("maskb", [128, 1024], BF16,
                           kind="ExternalInput").ap()
    out = nc.dram_tensor("out", [S, D], BF16, kind="ExternalOutput").ap()

    with tile.TileContext(nc) as tc, contextlib.ExitStack() as ctx:
        const = ctx.enter_context(tc.tile_pool(name="const", bufs=1))
        qt = const.tile([128, 2, S], BF16)     # Q^T/8 (+bq/8): chunk m = heads 2m,2m+1
        kt = const.tile([128, 2, S], BF16)     # K^T (+bk)
        va = const.tile([128, NT, CHA], BF16)  # V augmented: [s, head-major 65-col blocks]
        otn = const.tile([128, 2, S], BF16)    # normalized attention out, transposed
        mask_t = const.tile([128, 1024], BF16)
        ones1_t = const.tile([1, 128], BF16)
        onesf_t = const.tile([1, 64], F32)     # exp-table preload scratch
        bq_t = const.tile([128, 2], F32)
        bk_t = const.tile([128, 2], F32)
        bv_t = const.tile([1, CHA], BF16)
        wo_t = const.tile([128, 2, D], BF16)
        wor = wo.rearrange("(a p) n -> a p n", p=128)

        proj = ctx.enter_context(tc.tile_pool(name="proj", bufs=1))
        xt = proj.tile([128, 8, S], BF16)
        wq_t = proj.tile([128, 8, CH], BF16)
        wk_t = proj.tile([128, 8, CH], BF16)
        wv_t = proj.tile([128, 8, CHA], BF16)
        xTr = xT.rearrange("(a p) s -> a p s", p=128)
        wqr = wq.rearrange("(a p) c -> a p c", p=128)
        wkr = wk.rearrange("(a p) c -> a p c", p=128)
        wvr = wv.rearrange("(a p) c -> a p c", p=128)
        # DMA issue is ~0.6us per descriptor on one engine, so spread the
        # issue load over three engines: Sync takes chunks 0-3, ACT takes
        # chunks 6-7, GpSimd takes chunks 4-5 plus V weights and constants.
        def issue_chunk(eng, c, with_w=True):
            for k in range(4):
                sl = slice(k * 512, (k + 1) * 512)
                eng.dma_start(xt[:, c, sl], xTr[c][:, sl])
            if with_w:
                eng.dma_start(wq_t[:, c, :], wqr[c])
                eng.dma_start(wk_t[:, c, :], wkr[c])

        nc.sync.dma_start(onesf_t, onesf)
        nc.sync.dma_start(wq_t[:, 0, 0:128], wqr[0][:, 0:128])
        nc.sync.dma_start(wk_t[:, 0, 0:128], wkr[0][:, 0:128])
        # first x slice split small so the first matmul starts early
        nc.sync.dma_start(xt[:, 0, 0:256], xTr[0][:, 0:256])
        nc.sync.dma_start(xt[:, 0, 256:512], xTr[0][:, 256:512])
        for k in range(1, 4):
            sl = slice(k * 512, (k + 1) * 512)
            nc.sync.dma_start(xt[:, 0, sl], xTr[0][:, sl])
        nc.sync.dma_start(wq_t[:, 0, 128:CH], wqr[0][:, 128:CH])
        nc.sync.dma_start(wk_t[:, 0, 128:CH], wkr[0][:, 128:CH])
        for c in (3, 6, 7):
            issue_chunk(nc.sync, c)
        nc.sync.dma_start(bq_t, bq)
        nc.sync.dma_start(bk_t, bk)

        # preload the ACT exp table set before ACT's DMA issues
        nc.scalar.activation(onesf_t, onesf_t, AF.Exp)
        for c in (1, 2):
            issue_chunk(nc.scalar, c)

        for c in (4, 5):
            issue_chunk(nc.gpsimd, c)
        for c in range(8):
            nc.gpsimd.dma_start(wv_t[:, c, :], wvr[c])
        nc.gpsimd.dma_start(bv_t, bv)
        nc.gpsimd.dma_start(ones1_t, ones1)
        nc.gpsimd.dma_start(mask_t, maskb)
        for c2 in range(2):
            nc.gpsimd.dma_start(wo_t[:, c2, :], wor[c2])

        # ---- phase 1: chunk-0 projections + V -------------------------
        with tc.tile_pool(name="pps", bufs=8, space="PSUM") as pps:
            # Q and K chunk 0 interleaved per contraction chunk c so DMA
            # arrivals are consumed in order; 8 PSUM banks live at once.
            psq = [pps.tile([128, 512], F32, tag="ps", name=f"psq{n}")
                   for n in range(NQ)]
            psk = [pps.tile([128, 512], F32, tag="ps", name=f"psk{n}")
                   for n in range(NQ)]
            for c in range(8):
                for n in range(NQ):
                    nc.tensor.matmul(
                        psq[n], (wq_t[:, c, 0:128]),
                        (xt[:, c, n * 512:(n + 1) * 512]),
                        start=(c == 0), stop=(c == 7))
                for n in range(NQ):
                    nc.tensor.matmul(
                        psk[n], (wk_t[:, c, 0:128]),
                        (xt[:, c, n * 512:(n + 1) * 512]),
                        start=(c == 0), stop=(c == 7))
            for n in range(NQ):
                nc.scalar.activation(
                    qt[:, 0, n * 512:(n + 1) * 512], psq[n], AF.Identity,
                    bias=bq_t[:, 0:1], scale=0.125)
            for n in range(NQ):
                nc.scalar.activation(
                    kt[:, 0, n * 512:(n + 1) * 512], psk[n], AF.Identity,
                    bias=bk_t[:, 0:1], scale=1.0)

            # V (not transposed): stationary = x^T tile, moving = wv_aug
            for t in range(NT):
                ps = pps.tile([128, 512], F32, tag="ps", name="psv")
                psv = ps[:, 0:CHA]
                for c in range(8):
                    nc.tensor.matmul(
                        psv, (xt[:, c, t * 128:(t + 1) * 128]),
                        (wv_t[:, c, :]), start=(c == 0), stop=False)
                nc.tensor.matmul(psv, (ones1_t), (bv_t), start=False,
                                 stop=True)
                nc.vector.tensor_copy(va[:, t, :], psv)

        # ---- phase 2+3: attention, m=1 projections, output proj --------
        with tc.tile_pool(name="sm", bufs=4) as sm, \
             tc.tile_pool(name="ost", bufs=4) as ost, \
             tc.tile_pool(name="stp", bufs=1, space="PSUM") as stp, \
             tc.tile_pool(name="pvp", bufs=1, space="PSUM") as pvp, \
             tc.tile_pool(name="bcp", bufs=1, space="PSUM") as bcp:

            def attn_group(j, p, g, nkb, qsl):
                st = [stp.tile([128, 1024], F32, tag=f"st{_hh}",
                               name=f"st{_hh}") for _hh in range(2)]
                # For a diagonal k-block (rel >= 0), queries left of rel are
                # entirely masked, so QK/PV skip those columns exactly.
                # hh-outer so head 0's matmuls never queue behind a wait on
                # head 1's exp (PE executes its queue strictly in order)
                for hh in range(2):  # packed rows 0-63/64-127
                    oh = hh * 64
                    for i in range(2):
                        kb = 2 * g + i
                        rel = max(kb * 128 - j * 512, 0)
                        nc.tensor.matmul(
                            st[hh][:, i * 512 + rel:(i + 1) * 512],
                            (kt[oh:oh + 64, p, kb * 128:(kb + 1) * 128]),
                            (qt[oh:oh + 64, p,
                                j * 512 + rel:(j + 1) * 512]),
                            start=True, stop=True)
                pt = [None, None]
                for hh in range(2):
                    pt[hh] = sm.tile([128, 1024], BF16, tag=f"pt{hh}",
                                     name=f"pt{hh}")
                    nc.scalar.activation(pt[hh], st[hh], AF.Exp)
                for i in range(2):
                    kb = 2 * g + i
                    rel = kb * 128 - j * 512
                    if rel >= 0:
                        # only the 128-wide staircase needs zeroing now;
                        # split between GpSimd (idle) and DVE by head
                        for hh in range(2):
                            eng = nc.gpsimd if hh == 0 else nc.vector
                            sl = pt[hh][:, i * 512 + rel:
                                        i * 512 + rel + 128]
                            eng.tensor_mul(sl, sl, mask_t[:, 512:640])
                return pt

            def attn_pv(j, p, g, nkb, pv, pt):
                for hh in range(2):
                    h = 2 * p + hh
                    for i in range(2):
                        kb = 2 * g + i
                        rel = max(kb * 128 - j * 512, 0)
                        nc.tensor.matmul(
                            pv[hh][:, rel:512],
                            (va[:, kb, h * 65:h * 65 + 65]),
                            (pt[hh][:, i * 512 + rel:(i + 1) * 512]),
                            start=(kb == 0), stop=(kb == nkb - 1),
                            skip_group_check=True)

            def attn_tail(j, p, pv, qsl):
                for hh in range(2):
                    oh = hh * 64
                    den = sm.tile([1, 512], F32, tag="den")
                    nc.vector.tensor_copy(den, pv[hh][64:65, :])
                    rec = sm.tile([1, 512], F32, tag="rec")
                    nc.vector.reciprocal_approx_fast(rec, den)
                    rec_bf = sm.tile([1, 512], BF16, tag="recb")
                    nc.vector.tensor_copy(rec_bf, rec)
                    # broadcast 1/den across 64 partitions via a tiny matmul
                    bc = bcp.tile([128, 512], F32, tag="bc", name="bc")
                    nc.tensor.matmul(bc[0:64, :], (ones1_t[:, 0:64]),
                                     (rec_bf), start=True, stop=True)
                    bcs = sm.tile([64, 512], F32, tag="bcs")
                    nc.vector.tensor_copy(bcs, bc[0:64, :])
                    nc.vector.tensor_mul(otn[oh:oh + 64, p, qsl],
                                         pv[hh][0:64, :], bcs)

            def attn_block(j, p, interleave=None):
                nkb = 4 * (j + 1)
                qsl = slice(j * 512, (j + 1) * 512)
                pv = [pvp.tile([65, 512], F32, tag=f"pv{_hh}",
                               name=f"pv{_hh}") for _hh in range(2)]
                for g in range(nkb // 2):
                    pt = attn_group(j, p, g, nkb, qsl)
                    if interleave is not None:
                        interleave(g)
                    attn_pv(j, p, g, nkb, pv, pt)
                attn_tail(j, p, pv, qsl)

            # first attention block (j=3, p=0) interleaved with the m=1
            # Q/K projection chunks (1-bank n-outer accumulation)
            with tc.tile_pool(name="pjb", bufs=1, space="PSUM") as pjb:
                def m1_slot(g):
                    # slot g: one 512-wide n-block of Q(m=1) or K(m=1)
                    qk = g // 4           # 0 -> Q, 1 -> K
                    n = g % 4
                    w_t, dst, bias_t, scale = (
                        (wq_t, qt, bq_t, 0.125) if qk == 0
                        else (wk_t, kt, bk_t, 1.0))
                    ps = pjb.tile([128, 512], F32, tag="pb", name="pb")
                    for c in range(8):
                        nc.tensor.matmul(
                            ps, (w_t[:, c, 128:CH]),
                            (xt[:, c, n * 512:(n + 1) * 512]),
                            start=(c == 0), stop=(c == 7))
                    nc.scalar.activation(
                        dst[:, 1, n * 512:(n + 1) * 512], ps, AF.Identity,
                        bias=bias_t[:, 1:2], scale=scale)

                attn_block(3, 0, interleave=m1_slot)

            with tc.tile_pool(name="php", bufs=1, space="PSUM") as php:
                def ph3_unit(j, u, split_dma=False):
                    # one output-projection tile: q-tile t, column half n;
                    # alternate between the php bank and the (idle between
                    # block tails) bc bank so consecutive units pipeline
                    t = 4 * j + u // 2
                    n = u % 2
                    pool, tag = (php, "ph3") if u % 2 == 0 else (bcp, "bc")
                    ps = pool.tile([128, 512], F32, tag=tag, name="ph3")
                    for c2 in range(2):
                        nc.tensor.matmul(
                            ps, (otn[:, c2, t * 128:(t + 1) * 128]),
                            (wo_t[:, c2, n * 512:(n + 1) * 512]),
                            start=(c2 == 0), stop=(c2 == 1))
                    so = ost.tile([128, 512], BF16, tag=f"so{n}",
                                  name=f"so{n}")
                    nc.vector.tensor_copy(so, ps)
                    nsp = 2 if split_dma else 1
                    w = 512 // nsp
                    # alternate issue engine so the final drain parallelizes
                    # (gpsimd, not ACT: ACT paces the exp pipeline)
                    eng = nc.sync if u % 2 == 0 else nc.gpsimd
                    for v in range(nsp):
                        eng.dma_start(
                            out[t * 128:(t + 1) * 128,
                                n * 512 + v * w:n * 512 + (v + 1) * w],
                            so[:, v * w:(v + 1) * w])

                # ph3 units for finished blocks sprinkle into later
                # attention blocks as PE filler (two alternating PSUM banks
                # so consecutive units pipeline without head-of-line stalls)
                attn_block(3, 1)
                attn_block(2, 0, interleave=lambda g: (
                    ph3_unit(3, g) if g < 6 else None))
                attn_block(2, 1, interleave=lambda g: (
                    ph3_unit(3, 6 + g) if g < 2 else None))
                attn_block(1, 0, interleave=lambda g: ph3_unit(2, g))
                attn_block(1, 1, interleave=lambda g: ph3_unit(2, 4 + g))

                def two_units(j0, base, g):
                    ph3_unit(j0, base + 2 * g)
                    ph3_unit(j0, base + 2 * g + 1)

                attn_block(0, 0, interleave=lambda g: two_units(1, 0, g))
                attn_block(0, 1, interleave=lambda g: two_units(1, 4, g))
                for u in range(8):
                    ph3_unit(0, u, split_dma=(u >= 4))

    nc.compile()
    return nc


def _mask_np():
    # staircase causal keep-mask: mask[kk, x] = 0 if x < 512+kk else 1
    xs = np.arange(1024)[None, :]
    ks = np.arange(128)[:, None]
    return np.where(xs < 512 + ks, np.float32(0.0), np.float32(1.0))


def build_in_maps(x, Wq, bq, Wk, bk, Wv, bv, Wo):
    bf = mybir.dt.np(BF16)
    mask_np = _mask_np().astype(bf)
    ones1_np = np.ones((1, 128), dtype=bf)
    onesf_np = np.ones((1, 64), dtype=np.float32)
    xT_b = [np.ascontiguousarray(x[b].T).astype(bf) for b in range(B)]
    in_maps = []
    for c in range(N_CORES):
        b, tp = divmod(c, TPG)
        sl = slice(tp * CH, (tp + 1) * CH)
        wv_aug = np.zeros((D, CHA), dtype=np.float32)
        bv_aug = np.zeros((1, CHA), dtype=np.float32)
        for h in range(HPC):
            hsl = slice(tp * CH + h * DH, tp * CH + (h + 1) * DH)
            wv_aug[:, h * 65:h * 65 + DH] = Wv[:, hsl]
            bv_aug[0, h * 65:h * 65 + DH] = bv[hsl]
            bv_aug[0, h * 65 + DH] = 1.0
        in_maps.append({
            "xT": xT_b[b],
            "wq": np.ascontiguousarray(Wq[:, sl]).astype(bf),
            "wk": np.ascontiguousarray(Wk[:, sl]).astype(bf),
            "wv": wv_aug.astype(bf),
            "wo": np.ascontiguousarray(Wo[sl, :]).astype(bf),
            "bq": (bq[sl].astype(np.float32) * 0.125).reshape(2, 128).T.copy(),
            "bk": bk[sl].astype(np.float32).reshape(2, 128).T.copy(),
            "bv": bv_aug.astype(bf),
            "ones1": ones1_np,
            "onesf": onesf_np,
            "maskb": mask_np,
        })
    return in_maps


def _get_program():
    global _PROG
    if _PROG is None:
        _PROG = _build_program()
    return _PROG


def kernel(x, mask, Wq, bq, Wk, bk, Wv, bv, Wo, bo):
    x = np.asarray(x, dtype=np.float32)
    mask = np.asarray(mask)
    Wq, Wk, Wv, Wo = (np.asarray(w, dtype=np.float32)
                      for w in (Wq, Wk, Wv, Wo))
    bq, bk, bv, bo = (np.asarray(b, dtype=np.float32)
                      for b in (bq, bk, bv, bo))
    causal = bool(
        np.array_equal(mask != 0,
                       np.tril(np.ones((S, S), dtype=bool))))
    if not causal:
        # Fallback for non-causal masks: exact host computation.
        q = (x @ Wq + bq).reshape(B, S, H, DH).transpose(0, 2, 1, 3)
        k = (x @ Wk + bk).reshape(B, S, H, DH).transpose(0, 2, 1, 3)
        v = (x @ Wv + bv).reshape(B, S, H, DH).transpose(0, 2, 1, 3)
        attn = np.einsum("bhqd,bhkd->bhqk", q, k) / np.sqrt(np.float32(DH))
        attn = np.where(mask == 0, np.float32(-1e9), attn)
        attn = attn - attn.max(axis=-1, keepdims=True)
        e = np.exp(attn)
        p = e / e.sum(axis=-1, keepdims=True)
        o = np.einsum("bhqk,bhkd->bhqd", p, v)
        o = o.transpose(0, 2, 1, 3).reshape(B, S, D)
        return (o @ Wo + bo).astype(np.float32)

    nc = _get_program()
    in_maps = build_in_maps(x, Wq, bq, Wk, bk, Wv, bv, Wo)
    res = run_bass_kernel_spmd(nc, in_maps, core_ids=list(range(N_CORES)))
    out = np.zeros((B, S, D), dtype=np.float32)
    for c in range(N_CORES):
        out[c // TPG] += res.results[c]["out"].astype(np.float32)
    out += bo.astype(np.float32)
    return out
